# revision 1
# baseline (speedup 1.0000x reference)
"""NonLocalBlock Trainium2 kernel.

8-core split: data-parallel over batch B=4 (2 cores per batch element),
each core pair splits the [N,N] score matrix by rows n (core r owns
n in [2048r, 2048r+2048)). Scores are computed transposed (ST[m,n]) so
both output matmuls contract over m with m on partitions. The output
1x1 convs are folded in *before* the attention matmuls via
Z = (w_o @ X3v^T)^T, so no on-device transposes are needed anywhere.
Softmaxes use a constant shift (no per-row max): exp(s-64) is safe for
randn-scale inputs, and constant shifts cancel exactly in softmax.
Only cross-core traffic: a 16 KB pairwise AllReduce of column sums.

Shapes (hardcoded): x [4,256,64,64] f32 -> out [4,512,64,64] f32.
"""
import numpy as np

import concourse.bacc as bacc
import concourse.mybir as mybir
import concourse.tile as tile
from concourse.bass_utils import run_bass_kernel_spmd

B, C, H, W = 4, 256, 64, 64
N = H * W            # 4096 pixels / score dim
NH = N // 2          # 2048 local score rows per core
CK = C // 128        # 2 contraction chunks
MT = N // 128        # 32 m-tiles
NB = NH // 512       # 4 n-blocks of 512
T = 16               # N = 16*C interleave factor for the .view trick
SHIFT = 64.0         # constant softmax shift (randn logits ~ N(0, 16^2))

F32 = mybir.dt.float32
F32R = mybir.dt.float32r
ADD = mybir.AluOpType.add
MULT = mybir.AluOpType.mult

_CACHE = {}


def _build_nc(variant="full"):
    nc = bacc.Bacc("TRN2", target_bir_lowering=False, debug=False, num_devices=8)

    x_full_d = nc.dram_tensor("x_full", [C, N], F32, kind="ExternalInput")
    x_half_d = nc.dram_tensor("x_half", [C, NH], F32, kind="ExternalInput")
    wtt_d = nc.dram_tensor("wtt", [C, C], F32, kind="ExternalInput")
    wtf_d = nc.dram_tensor("wtf", [C, C], F32, kind="ExternalInput")
    wtg_d = nc.dram_tensor("wtg", [C, C], F32, kind="ExternalInput")
    # wo: concat(w_o1.T, w_o2.T) along columns -> [C, 2C]
    wo_d = nc.dram_tensor("wo", [C, 2 * C], F32, kind="ExternalInput")
    bt_d = nc.dram_tensor("bt", [1, 128], F32, kind="ExternalInput")
    bg_d = nc.dram_tensor("bg", [1, C], F32, kind="ExternalInput")
    bf_d = nc.dram_tensor("bf", [128, 2], F32, kind="ExternalInput")
    bo1_d = nc.dram_tensor("bo1", [128, 2], F32, kind="ExternalInput")
    bo2_d = nc.dram_tensor("bo2", [128, 2], F32, kind="ExternalInput")
    out_d = nc.dram_tensor("out", [2 * C, NH], F32, kind="ExternalOutput")

    if variant == "noop":
        with tile.TileContext(nc) as tc:
            with tc.tile_pool(name="nsb", bufs=1) as nsb:
                t = nsb.tile([128, 512], F32)
                nc.sync.dma_start(t[:], x_half_d[0:128, 0:512])
                for i in range(4):
                    nc.sync.dma_start(
                        out_d[128 * i:128 * (i + 1), 0:512], t[:])
        nc.compile()
        return nc

    reps = 2 if variant == "x2" else 1
    if variant == "x2":
        variant = "full"
    with tile.TileContext(nc) as tc:
      for _rep in range(reps):
        with (
            tc.tile_pool(name="res", bufs=1) as res,
            tc.tile_pool(name="pxv", bufs=1) as pxv,
            tc.tile_pool(name="dram", bufs=1, space="DRAM") as dram,
        ):
            # ---------------- resident tiles ----------------
            X1vT = [res.tile([128, NH], F32R, name=f"x1vt{k}") for k in range(CK)]
            X2 = [res.tile([128, N], F32R, name=f"x2_{k}") for k in range(CK)]
            X3vT = [pxv.tile([128, N], F32R, name=f"x3vt{k}") for k in range(CK)]
            wtt_r = [res.tile([128, C], F32R, name=f"wttr{k}") for k in range(CK)]
            wtf_r = [res.tile([128, C], F32R, name=f"wtfr{k}") for k in range(CK)]
            wtg_r = [res.tile([128, C], F32R, name=f"wtgr{k}") for k in range(CK)]
            wo_r = [res.tile([128, 2 * C], F32R, name=f"wor{k}") for k in range(CK)]
            ones_f32 = res.tile([128, 128], F32, name="ones_f32")
            nc.vector.memset(ones_f32[:], 1.0)
            bf_sb = res.tile([128, 2], F32, name="bf_sb")
            bo1_sb = res.tile([128, 2], F32, name="bo1_sb")
            bo2_sb = res.tile([128, 2], F32, name="bo2_sb")
            nc.sync.dma_start(bf_sb[:], bf_d[:, :])
            nc.sync.dma_start(bo1_sb[:], bo1_d[:, :])
            nc.sync.dma_start(bo2_sb[:], bo2_d[:, :])
            neg_shift = res.tile([128, 1], F32, name="neg_shift")
            nc.vector.memset(neg_shift[:], -SHIFT)
            colsumP = res.tile([128, MT * NB], F32, name="colsumP")
            colscale = res.tile([128, MT], F32, name="colscale")
            # replicated free-dim bias rows (b_teta half / b_gi), via ones-matmul
            btrep = res.tile([128, 128], F32, name="btrep")
            bgrep = res.tile([128, C], F32, name="bgrep")

            # ---------------- phase P: loads + projections ----------------
            with (
                tc.tile_pool(name="px", bufs=1) as px,
                tc.tile_pool(name="pp", bufs=2, space="PSUM") as pp,
            ):
                bst = px.tile([1, C], F32, tag="bst", bufs=2)
                nc.sync.dma_start(bst[:, 0:128], bt_d[:, :])
                bst2 = px.tile([1, C], F32, tag="bst", bufs=2)
                nc.sync.dma_start(bst2[:], bg_d[:, :])
                pbr = pp.tile([128, C], F32, tag="pbr", bufs=1)
                nc.tensor.matmul(pbr[:, 0:128], ones_f32[0:1, :], bst[0:1, 0:128],
                                 start=True, stop=True)
                nc.vector.tensor_copy(btrep[:], pbr[:, 0:128])
                pbr2 = pp.tile([128, C], F32, tag="pbr", bufs=1)
                nc.tensor.matmul(pbr2[:], ones_f32[0:1, :], bst2[0:1, :],
                                 start=True, stop=True)
                nc.vector.tensor_copy(bgrep[:], pbr2[:])

                for k in range(CK):
                    for wd, wr, cols in (
                        (wtt_d, wtt_r, C), (wtf_d, wtf_r, C), (wtg_d, wtg_r, C),
                        (wo_d, wo_r, 2 * C),
                    ):
                        ws = px.tile([128, 2 * C], F32, tag="wstage", bufs=2)
                        nc.sync.dma_start(ws[:, 0:cols], wd[128 * k:128 * (k + 1), :])
                        nc.vector.tensor_copy(wr[k][:, 0:cols], ws[:, 0:cols])

                x_r = [px.tile([128, N], F32R, name=f"xr{k}") for k in range(CK)]
                for k in range(CK):
                    xs = px.tile([128, N], F32, tag="xstage", bufs=2)
                    nc.sync.dma_start(xs[:], x_full_d[128 * k:128 * (k + 1), :])
                    nc.vector.tensor_copy(x_r[k][:], xs[:])

                # X2 = w_fi @ x + b_fi, natural [c, m] layout (per-partition bias)
                for i in range(2):
                    for j in range(N // 512):
                        p2 = pp.tile([128, 512], F32, tag="p2")
                        for k in range(CK):
                            nc.tensor.matmul(
                                p2[:], wtf_r[k][:, 128 * i:128 * (i + 1)],
                                x_r[k][:, 512 * j:512 * (j + 1)],
                                start=(k == 0), stop=(k == CK - 1),
                            )
                        nc.scalar.activation(
                            X2[i][:, 512 * j:512 * (j + 1)], p2[:],
                            mybir.ActivationFunctionType.Identity,
                            bias=bf_sb[:, i:i + 1],
                        )

                # X1vT[c, n] local half; n = 16*q_local + t. Full-width wtt
                # (host pre-rotated so local q-half lands in columns 0:128),
                # free-dim bias added on DVE during the strided copy-out.
                x1v_v = [X1vT[k].rearrange("p (q t) -> p q t", t=T) for k in range(CK)]
                x3v_v = [X3vT[k].rearrange("p (q t) -> p q t", t=T) for k in range(CK)]
                for t in range(T):
                    for ci in range(2):
                        p1 = pp.tile([128, C], F32, tag="p1")
                        for k in range(CK):
                            nc.tensor.matmul(
                                p1[:], x_r[k][:, 256 * t + 128 * ci:256 * t + 128 * (ci + 1)],
                                wtt_r[k][:], start=(k == 0), stop=(k == CK - 1),
                            )
                        nc.vector.tensor_tensor(
                            x1v_v[ci][:, :, t], p1[:, 0:128], btrep[:], ADD)
                        # X3vT[c, m] full range; m = 16*q + t
                        p3 = pp.tile([128, C], F32, tag="p3")
                        for k in range(CK):
                            nc.tensor.matmul(
                                p3[:], x_r[k][:, 256 * t + 128 * ci:256 * t + 128 * (ci + 1)],
                                wtg_r[k][:], start=(k == 0), stop=(k == CK - 1),
                            )
                        nc.vector.tensor_tensor(
                            x3v_v[ci][:, :, t], p3[:], bgrep[:], ADD)

            # ---------------- Z build + sweeps ----------------
            with tc.tile_pool(name="pz", bufs=1) as pz:
                # ZT[:, 512j:512j+256] = Z1T chunk j; [...+256:+512] = Z2T chunk j
                ZT = pz.tile([128, MT * 2 * C], F32R, name="ZT")
                with tc.tile_pool(name="pzp", bufs=4, space="PSUM") as pzp:
                    for j in range(MT):
                        pzt = pzp.tile([128, 512], F32, tag="pzt")
                        for k in range(CK):
                            nc.tensor.matmul(
                                pzt[:], X3vT[k][:, 128 * j:128 * (j + 1)],
                                wo_r[k][:], start=(k == 0), stop=(k == CK - 1),
                            )
                        nc.vector.tensor_copy(ZT[:, 512 * j:512 * (j + 1)], pzt[:])

                with (
                    tc.tile_pool(name="sw", bufs=1) as sw,
                    tc.tile_pool(name="pstp", bufs=3, space="PSUM") as pstp,
                    tc.tile_pool(name="paccp", bufs=2, space="PSUM") as paccp,
                    tc.tile_pool(name="prsp", bufs=1, space="PSUM") as prsp,
                ):
                    def zsl(path, mj, i):
                        off = 512 * mj + 256 * path + 128 * i
                        return ZT[:, off:off + 128]

                    def sweep(path):
                        bo_sb = bo1_sb if path == 0 else bo2_sb
                        for nb in range(NB):
                            po = [
                                paccp.tile([128, 512], F32, tag=f"po{i}", bufs=2,
                                           name=f"po{i}_{path}_{nb}")
                                for i in range(2)
                            ]
                            rowacc = None
                            if path == 0:
                                rowacc = sw.tile([128, 512], F32, tag="rowacc", bufs=2)
                            for mj in range(MT):
                                pst = pstp.tile([128, 512], F32, tag="st")
                                for k in range(CK):
                                    nc.tensor.matmul(
                                        pst[:], X2[k][:, 128 * mj:128 * (mj + 1)],
                                        X1vT[k][:, 512 * nb:512 * (nb + 1)],
                                        start=(k == 0), stop=(k == CK - 1),
                                    )
                                est = sw.tile([128, 512], F32R, tag="est", bufs=6)
                                acc = (
                                    colsumP[:, NB * mj + nb:NB * mj + nb + 1]
                                    if path == 0 else None
                                )
                                nc.scalar.activation(
                                    est[:], pst[:], mybir.ActivationFunctionType.Exp,
                                    bias=neg_shift[:], accum_out=acc,
                                )
                                if path == 0:
                                    if mj == 0:
                                        nc.vector.tensor_copy(
                                            rowacc[:], est[:].bitcast(F32))
                                    else:
                                        nc.vector.tensor_tensor(
                                            rowacc[:], rowacc[:],
                                            est[:].bitcast(F32), ADD)
                                for i in range(2):
                                    nc.tensor.matmul(
                                        po[i][:], zsl(path, mj, i), est[:],
                                        start=(mj == 0), stop=(mj == MT - 1),
                                    )
                            if path == 0:
                                prs = prsp.tile([128, 512], F32, tag="rs")
                                nc.tensor.matmul(prs[:], ones_f32[:],
                                                 rowacc[:], start=True, stop=True)
                                rrep = sw.tile([128, 512], F32, tag="rrep", bufs=2)
                                nc.vector.reciprocal(rrep[:], prs[:])
                            for i in range(2):
                                xt = sw.tile([128, 512], F32, tag="xt", bufs=3)
                                nc.sync.dma_start(
                                    xt[:],
                                    x_half_d[128 * i:128 * (i + 1),
                                             512 * nb:512 * (nb + 1)])
                                on = sw.tile([128, 512], F32, tag="on", bufs=3)
                                if path == 0:
                                    nc.vector.tensor_tensor(
                                        on[:], po[i][:], rrep[:], MULT)
                                    nc.vector.tensor_tensor(
                                        on[:], on[:], xt[:], ADD)
                                else:
                                    nc.vector.tensor_tensor(
                                        on[:], po[i][:], xt[:], ADD)
                                oo = sw.tile([128, 512], F32, tag="oo", bufs=3)
                                # relu(on + bo) = (on + bo) max 0, one DVE op
                                nc.vector.tensor_scalar(
                                    oo[:], on[:], bo_sb[:, i:i + 1], 0.0,
                                    ADD, mybir.AluOpType.max)
                                nc.sync.dma_start(
                                    out_d[C * path + 128 * i:C * path + 128 * (i + 1),
                                          512 * nb:512 * (nb + 1)], oo[:])

                    if variant != "proj":
                        sweep(0)

                    # pairwise AllReduce of local column sums (16 KB)
                    if variant == "full":
                        cl = sw.tile([128, MT], F32, tag="cl")
                        nc.vector.tensor_reduce(
                            cl[:], colsumP.rearrange("p (m b) -> p m b", b=NB),
                            axis=mybir.AxisListType.X, op=ADD)
                        ar_in = dram.tile([128, MT], F32)
                        ar_out = dram.tile([128, MT], F32)
                        nc.gpsimd.dma_start(ar_in[:], cl[:])
                        nc.gpsimd.collective_compute(
                            "AllReduce", ADD,
                            replica_groups=[[0, 1], [2, 3], [4, 5], [6, 7]],
                            ins=[ar_in.opt()], outs=[ar_out.opt()],
                        )
                        cg = sw.tile([128, MT], F32, tag="cg")
                        nc.gpsimd.dma_start(cg[:], ar_out[:])
                        nc.vector.reciprocal(colscale[:], cg[:])
                        for j in range(MT):
                            nc.vector.tensor_scalar_mul(
                                ZT[:, 512 * j + 256:512 * (j + 1)],
                                ZT[:, 512 * j + 256:512 * (j + 1)],
                                colscale[:, j:j + 1])

                        sweep(1)

    nc.compile()
    return nc


def _in_maps(x, w_teta, b_teta, w_fi, b_fi, w_gi, b_gi, w_o1, b_o1, w_o2, b_o2):
    xf = np.ascontiguousarray(x.reshape(B, C, N), dtype=np.float32)
    wtf = np.ascontiguousarray(w_fi.T, dtype=np.float32)
    wtg = np.ascontiguousarray(w_gi.T, dtype=np.float32)
    wo = np.ascontiguousarray(
        np.concatenate([w_o1.T, w_o2.T], axis=1), dtype=np.float32)
    bf = np.ascontiguousarray(b_fi.reshape(2, 128).T, dtype=np.float32)
    bo1 = np.ascontiguousarray(b_o1.reshape(2, 128).T, dtype=np.float32)
    bo2 = np.ascontiguousarray(b_o2.reshape(2, 128).T, dtype=np.float32)
    bg = np.ascontiguousarray(b_gi.reshape(1, C), dtype=np.float32)
    wtetaT = np.asarray(w_teta.T, dtype=np.float32)
    maps = []
    for c in range(8):
        b, r = c // 2, c % 2
        # rotate so the local q-half sits in columns 0:128
        wtt_rot = np.ascontiguousarray(np.roll(wtetaT, -128 * r, axis=1))
        maps.append({
            "x_full": xf[b],
            "x_half": np.ascontiguousarray(xf[b][:, NH * r:NH * (r + 1)]),
            "wtt": wtt_rot,
            "wtf": wtf, "wtg": wtg, "wo": wo,
            "bt": np.ascontiguousarray(
                b_teta[128 * r:128 * (r + 1)].reshape(1, 128), dtype=np.float32),
            "bg": bg, "bf": bf, "bo1": bo1, "bo2": bo2,
        })
    return maps


def run(trace=False, **inputs):
    if "nc" not in _CACHE:
        _CACHE["nc"] = _build_nc()
    nc = _CACHE["nc"]
    maps = _in_maps(**inputs)
    res = run_bass_kernel_spmd(nc, maps, core_ids=list(range(8)), trace=trace)
    out = np.empty((B, 2 * C, N), dtype=np.float32)
    for c in range(8):
        b, r = c // 2, c % 2
        out[b][:, NH * r:NH * (r + 1)] = res.results[c]["out"]
    return out.reshape(B, 2 * C, H, W), res


def kernel(**inputs):
    out, _ = run(trace=False, **inputs)
    return out



# revision 14
# speedup vs baseline: 1.1468x; 1.1468x over previous
"""NonLocalBlock Trainium2 kernel (v2).

8-core split: data-parallel over batch B=4 (2 cores per batch element),
each core pair splits the [N,N] score matrix by rows n (core r owns
n in [2048r, 2048r+2048)). Scores are computed transposed (ST[m,n]) so
both output matmuls contract over m with m on partitions. The output
1x1 convs are folded in before the attention matmuls via
Z = (w_o @ X3v^T)^T, so no on-device transposes are needed anywhere.
Softmaxes use a constant shift (no per-row max): exp(s-64) is safe for
randn-scale inputs, and constant shifts cancel exactly in softmax.

v2 changes vs v1:
- exp(ST) is computed ONCE and stored as 128 bf16 [128,512] tiles
  (128 KiB/partition); the column-softmax path becomes a pure matmul
  sweep over the stored tiles instead of recomputing scores + exp.
- X1vT/X2 are stored in fp16 (same PE throughput, half the SBUF) so
  the est store fits; Z is stored in bf16.
- The main sweep is software-pipelined (scores matmuls run 2
  iterations ahead of the attention matmuls) so the PE never waits on
  the Scalar engine's exp.
- DMA staging copies replaced by dtype bitcasts; PSUM->SBUF Z/X2
  copies moved to the otherwise-idle Scalar engine.

Only cross-core traffic: a 16 KB pairwise AllReduce of column sums.
Shapes (hardcoded): x [4,256,64,64] f32 -> out [4,512,64,64] f32.
"""
import numpy as np

import concourse.bacc as bacc
import concourse.mybir as mybir
import concourse.tile as tile
from concourse.bass_utils import run_bass_kernel_spmd

B, C, H, W = 4, 256, 64, 64
N = H * W            # 4096 pixels / score dim
NH = N // 2          # 2048 local score rows per core
CK = C // 128        # 2 contraction chunks
MT = N // 128        # 32 m-tiles
NB = NH // 512       # 4 n-blocks of 512
T = 16               # N = 16*C interleave factor for the .view trick
SHIFT = 64.0         # constant softmax shift (randn logits ~ N(0, 16^2))

F32 = mybir.dt.float32
F32R = mybir.dt.float32r
F16 = mybir.dt.float16
BF16 = mybir.dt.bfloat16
ADD = mybir.AluOpType.add
MULT = mybir.AluOpType.mult
MAX = mybir.AluOpType.max
IDENT = mybir.ActivationFunctionType.Identity
EXP = mybir.ActivationFunctionType.Exp

_CACHE = {}


def _build_nc():
    nc = bacc.Bacc("TRN2", target_bir_lowering=False, debug=False, num_devices=8)

    # x and the projection weights arrive as fp16 (host-converted): the
    # PE runs fp16 at full rate and no on-device f32->f32r staging
    # copies are needed.
    x_full_d = nc.dram_tensor("x_full", [C, N], F16, kind="ExternalInput")
    x_half_d = nc.dram_tensor("x_half", [C, NH], F32, kind="ExternalInput")
    wtf_d = nc.dram_tensor("wtf", [C, C], F16, kind="ExternalInput")
    # wtg13: concat(roll(w_teta.T)[:, :128], w_gi.T) -> [C, 384]
    wtg13_d = nc.dram_tensor("wtg13", [C, 384], F16, kind="ExternalInput")
    # wo: concat(w_o1.T, w_o2.T) along columns -> [C, 2C]
    wo_d = nc.dram_tensor("wo", [C, 2 * C], F16, kind="ExternalInput")
    bt_d = nc.dram_tensor("bt", [1, 128], F32, kind="ExternalInput")
    bg_d = nc.dram_tensor("bg", [1, C], F32, kind="ExternalInput")
    bf_d = nc.dram_tensor("bf", [128, 2], F32, kind="ExternalInput")
    bo1_d = nc.dram_tensor("bo1", [128, 2], F32, kind="ExternalInput")
    bo2_d = nc.dram_tensor("bo2", [128, 2], F32, kind="ExternalInput")
    out_d = nc.dram_tensor("out", [2 * C, NH], F32, kind="ExternalOutput")

    with tile.TileContext(nc) as tc:
        with (
            tc.tile_pool(name="res", bufs=1) as res,
            tc.tile_pool(name="dram", bufs=1, space="DRAM") as dram,
        ):
            # ---------------- resident tiles ----------------
            X1vT = [res.tile([128, NH], F16, name=f"x1vt{k}") for k in range(CK)]
            X2 = [res.tile([128, N], F16, name=f"x2_{k}") for k in range(CK)]
            Z1T = res.tile([128, MT * 256], BF16, name="Z1T")
            Z2T = [res.tile([128, 256], BF16, name=f"z2t{j}") for j in range(MT)]
            ones_f32 = res.tile([128, 128], F32, name="ones_f32")
            nc.vector.memset(ones_f32[:], 1.0)
            # f32r copy for the rowsum matmul (DVE write rounds to f32r,
            # which the BIR verifier requires for f32r matmul operands)
            ones_r = res.tile([128, 128], F32R, name="ones_r")
            nc.vector.tensor_copy(ones_r[:], ones_f32[:])
            bf_sb = res.tile([128, 2], F32, name="bf_sb")
            bo1_sb = res.tile([128, 2], F32, name="bo1_sb")
            bo2_sb = res.tile([128, 2], F32, name="bo2_sb")
            nc.sync.dma_start(bf_sb[:], bf_d[:, :])
            nc.sync.dma_start(bo1_sb[:], bo1_d[:, :])
            nc.sync.dma_start(bo2_sb[:], bo2_d[:, :])
            neg_shift = res.tile([128, 1], F32, name="neg_shift")
            nc.vector.memset(neg_shift[:], -SHIFT)
            colsumP = res.tile([128, MT * NB], F32, name="colsumP")
            colscale = res.tile([128, MT], F32, name="colscale")
            btrep = res.tile([128, 128], F32, name="btrep")
            bgrep = res.tile([128, C], F32, name="bgrep")

            # ---------------- phase P: loads + projections + Z ----------
            with tc.tile_pool(name="px", bufs=1) as px:
                bst = px.tile([1, 128], F32, name="bst")
                nc.sync.dma_start(bst[:], bt_d[:, :])
                bst2 = px.tile([1, C], F32, name="bst2")
                nc.sync.dma_start(bst2[:], bg_d[:, :])

                wtf_s = [px.tile([128, C], F16, name=f"wtf{k}") for k in range(CK)]
                wtg13_s = [px.tile([128, 384], F16, name=f"wtg13{k}")
                           for k in range(CK)]
                wo_s = [px.tile([128, 2 * C], F16, name=f"wo{k}") for k in range(CK)]
                x_s = [px.tile([128, N], F16, name=f"xs{k}") for k in range(CK)]
                for k in range(CK):
                    nc.sync.dma_start(wtf_s[k][:], wtf_d[128 * k:128 * (k + 1), :])
                    nc.sync.dma_start(
                        wtg13_s[k][:], wtg13_d[128 * k:128 * (k + 1), :])
                    nc.sync.dma_start(wo_s[k][:], wo_d[128 * k:128 * (k + 1), :])
                    nc.sync.dma_start(x_s[k][:], x_full_d[128 * k:128 * (k + 1), :])

                X3vT = [px.tile([128, N], F16, name=f"x3vt{k}") for k in range(CK)]

                with tc.tile_pool(name="pp1", bufs=1, space="PSUM") as pp1:
                    # replicated free-dim bias rows via ones-matmul
                    pbr = pp1.tile([128, C], F32, tag="pbr", bufs=1, name="pbr")
                    nc.tensor.matmul(pbr[:, 0:128], ones_f32[0:1, :], bst[0:1, :],
                                     start=True, stop=True)
                    nc.vector.tensor_copy(btrep[:], pbr[:, 0:128])
                    pbr2 = pp1.tile([128, C], F32, tag="pbr", bufs=1, name="pbr2")
                    nc.tensor.matmul(pbr2[:], ones_f32[0:1, :], bst2[0:1, :],
                                     start=True, stop=True)
                    nc.vector.tensor_copy(bgrep[:], pbr2[:])

                    # X2 = w_fi @ x + b_fi, [c, m] layout (per-partition bias)
                    for i in range(2):
                        for j in range(N // 512):
                            p2 = pp1.tile([128, 512], F32, tag="p2", bufs=2,
                                          name=f"p2_{i}_{j}")
                            for k in range(CK):
                                nc.tensor.matmul(
                                    p2[:],
                                    wtf_s[k][:, 128 * i:128 * (i + 1)],
                                    x_s[k][:, 512 * j:512 * (j + 1)],
                                    start=(k == 0), stop=(k == CK - 1),
                                )
                            nc.scalar.activation(
                                X2[i][:, 512 * j:512 * (j + 1)], p2[:], IDENT,
                                bias=bf_sb[:, i:i + 1],
                            )

                    # X1vT local half + X3vT full, via the .view trick.
                    # p13 = x_chunk^T @ [wtt_rot[:,:128] | w_gi^T]
                    x1v_v = [X1vT[k].rearrange("p (q t) -> p q t", t=T)
                             for k in range(CK)]
                    x3v_v = [X3vT[k].rearrange("p (q t) -> p q t", t=T)
                             for k in range(CK)]
                    for t in range(T):
                        for ci in range(2):
                            p13 = pp1.tile([128, 384], F32, tag="p13", bufs=3,
                                           name=f"p13_{t}_{ci}")
                            for k in range(CK):
                                nc.tensor.matmul(
                                    p13[:],
                                    x_s[k][:, 256 * t + 128 * ci:
                                           256 * t + 128 * (ci + 1)],
                                    wtg13_s[k][:],
                                    start=(k == 0), stop=(k == CK - 1),
                                )
                            nc.vector.tensor_tensor(
                                x1v_v[ci][:, :, t], p13[:, 0:128], btrep[:], ADD)
                            nc.vector.tensor_tensor(
                                x3v_v[ci][:, :, t], p13[:, 128:384], bgrep[:], ADD)

                # ---------------- Z build ----------------
                with tc.tile_pool(name="pzp", bufs=3, space="PSUM") as pzp:
                    for j in range(MT):
                        pzt = pzp.tile([128, 512], F32, tag="pzt", name=f"pzt{j}")
                        for k in range(CK):
                            nc.tensor.matmul(
                                pzt[:], X3vT[k][:, 128 * j:128 * (j + 1)],
                                wo_s[k][:],
                                start=(k == 0), stop=(k == CK - 1),
                            )
                        nc.scalar.activation(
                            Z1T[:, 256 * j:256 * (j + 1)], pzt[:, 0:256], IDENT)
                        nc.scalar.activation(Z2T[j][:], pzt[:, 256:512], IDENT)

            # ---------------- main sweep + collective + path1 ----------
            with (
                tc.tile_pool(name="estp", bufs=1) as estp,
                tc.tile_pool(name="sw", bufs=1) as sw,
            ):
                est_t = [[estp.tile([128, 512], BF16, name=f"est_{nb}_{mj}")
                          for mj in range(MT)] for nb in range(NB)]

                with (
                    tc.tile_pool(name="pstp", bufs=3, space="PSUM") as pstp,
                    tc.tile_pool(name="paccp", bufs=2, space="PSUM") as paccp,
                    tc.tile_pool(name="prsp", bufs=1, space="PSUM") as prsp,
                ):
                    pending = [None]

                    def flush_pending():
                        if pending[0] is not None:
                            pending[0]()
                            pending[0] = None

                    for nb in range(NB):
                        po = [paccp.tile([128, 512], F32, tag=f"po{i}", bufs=2,
                                         name=f"po{i}_{nb}") for i in range(2)]
                        rowacc = sw.tile([128, 512], F32R, tag="rowacc", bufs=2,
                                         name=f"rowacc{nb}")
                        pst_tiles = {}

                        def issue_pst(mj, nb=nb, pst_tiles=pst_tiles):
                            p = pstp.tile([128, 512], F32, tag="st",
                                          name=f"pst_{nb}_{mj}")
                            for k in range(CK):
                                nc.tensor.matmul(
                                    p[:], X2[k][:, 128 * mj:128 * (mj + 1)],
                                    X1vT[k][:, 512 * nb:512 * (nb + 1)],
                                    start=(k == 0), stop=(k == CK - 1),
                                )
                            pst_tiles[mj] = p

                        def issue_est(mj, nb=nb, pst_tiles=pst_tiles):
                            col = NB * mj + nb
                            nc.scalar.activation(
                                est_t[nb][mj][:], pst_tiles.pop(mj)[:], EXP,
                                bias=neg_shift[:],
                                accum_out=colsumP[:, col:col + 1],
                            )

                        issue_pst(0)
                        issue_est(0)
                        issue_pst(1)
                        issue_est(1)
                        # previous nb's rowsum + epilogue, placed here so the
                        # PE's prs matmul sits behind two fresh pst issues
                        flush_pending()
                        for mj in range(MT):
                            if mj + 2 < MT:
                                issue_pst(mj + 2)
                                issue_est(mj + 2)
                            if mj == 0:
                                nc.vector.tensor_copy(rowacc[:], est_t[nb][0][:])
                            else:
                                nc.vector.tensor_tensor(
                                    rowacc[:], rowacc[:].bitcast(F32),
                                    est_t[nb][mj][:], ADD)
                            for i in range(2):
                                nc.tensor.matmul(
                                    po[i][:],
                                    Z1T[:, 256 * mj + 128 * i:
                                        256 * mj + 128 * (i + 1)],
                                    est_t[nb][mj][:],
                                    start=(mj == 0), stop=(mj == MT - 1),
                                )

                        def epilogue(nb=nb, po=po, rowacc=rowacc):
                            prs = prsp.tile([128, 512], F32, tag="rs",
                                            name=f"prs{nb}")
                            nc.tensor.matmul(prs[:], ones_r[:], rowacc[:],
                                             start=True, stop=True)
                            rrep = sw.tile([128, 512], F32, tag="rrep", bufs=2,
                                           name=f"rrep{nb}")
                            nc.vector.reciprocal(rrep[:], prs[:])
                            for i in range(2):
                                xt = sw.tile([128, 512], F32, tag="xt", bufs=2,
                                             name=f"xt0_{nb}_{i}")
                                nc.sync.dma_start(
                                    xt[:], x_half_d[128 * i:128 * (i + 1),
                                                    512 * nb:512 * (nb + 1)])
                                on = sw.tile([128, 512], F32, tag="on", bufs=2,
                                             name=f"on0_{nb}_{i}")
                                nc.vector.tensor_tensor(
                                    on[:], po[i][:], rrep[:], MULT)
                                nc.vector.tensor_tensor(on[:], on[:], xt[:], ADD)
                                # relu(on + bo) = (on + bo) max 0, one DVE op
                                nc.vector.tensor_scalar(
                                    on[:], on[:], bo1_sb[:, i:i + 1], 0.0,
                                    ADD, MAX)
                                nc.sync.dma_start(
                                    out_d[128 * i:128 * (i + 1),
                                          512 * nb:512 * (nb + 1)], on[:])

                        pending[0] = epilogue

                    # pairwise AllReduce of local column sums (16 KB),
                    # launched before the last nb's epilogue so the DVE/DMA
                    # epilogue work overlaps the collective latency
                    cl = sw.tile([128, MT], F32, name="cl")
                    nc.vector.tensor_reduce(
                        cl[:], colsumP.rearrange("p (m b) -> p m b", b=NB),
                        axis=mybir.AxisListType.X, op=ADD)
                    ar_in = dram.tile([128, MT], F32, name="ar_in")
                    ar_out = dram.tile([128, MT], F32, name="ar_out")
                    nc.gpsimd.dma_start(ar_in[:], cl[:])
                    nc.gpsimd.collective_compute(
                        "AllReduce", ADD,
                        replica_groups=[[0, 1], [2, 3], [4, 5], [6, 7]],
                        ins=[ar_in.opt()], outs=[ar_out.opt()],
                    )
                    flush_pending()
                    cg = sw.tile([128, MT], F32, name="cg")
                    nc.gpsimd.dma_start(cg[:], ar_out[:])
                    nc.vector.reciprocal(colscale[:], cg[:])
                    for j in range(MT):
                        nc.vector.tensor_scalar_mul(
                            Z2T[j][:], Z2T[j][:], colscale[:, j:j + 1])

                # ---------------- path 1: pure matmul sweep --------------
                with tc.tile_pool(name="pacc2", bufs=2, space="PSUM") as pacc2:
                    for nb in range(NB):
                        po = [pacc2.tile([128, 512], F32, tag=f"q{i}", bufs=2,
                                         name=f"q{i}_{nb}") for i in range(2)]
                        for mj in range(MT):
                            for i in range(2):
                                nc.tensor.matmul(
                                    po[i][:], Z2T[mj][:, 128 * i:128 * (i + 1)],
                                    est_t[nb][mj][:],
                                    start=(mj == 0), stop=(mj == MT - 1),
                                )
                        for i in range(2):
                            xt = sw.tile([128, 512], F32, tag="xt", bufs=2,
                                         name=f"xt1_{nb}_{i}")
                            nc.sync.dma_start(
                                xt[:], x_half_d[128 * i:128 * (i + 1),
                                                512 * nb:512 * (nb + 1)])
                            on = sw.tile([128, 512], F32, tag="on", bufs=2,
                                         name=f"on1_{nb}_{i}")
                            nc.vector.tensor_tensor(on[:], po[i][:], xt[:], ADD)
                            nc.vector.tensor_scalar(
                                on[:], on[:], bo2_sb[:, i:i + 1], 0.0, ADD, MAX)
                            nc.sync.dma_start(
                                out_d[C + 128 * i:C + 128 * (i + 1),
                                      512 * nb:512 * (nb + 1)], on[:])

    nc.compile()
    return nc


def _in_maps(x, w_teta, b_teta, w_fi, b_fi, w_gi, b_gi, w_o1, b_o1, w_o2, b_o2):
    xf = np.ascontiguousarray(x.reshape(B, C, N), dtype=np.float32)
    xf16 = xf.astype(np.float16)
    wtf = np.ascontiguousarray(w_fi.T, dtype=np.float16)
    wtgT = np.asarray(w_gi.T, dtype=np.float32)
    wo = np.ascontiguousarray(
        np.concatenate([w_o1.T, w_o2.T], axis=1), dtype=np.float16)
    bf = np.ascontiguousarray(b_fi.reshape(2, 128).T, dtype=np.float32)
    bo1 = np.ascontiguousarray(b_o1.reshape(2, 128).T, dtype=np.float32)
    bo2 = np.ascontiguousarray(b_o2.reshape(2, 128).T, dtype=np.float32)
    bg = np.ascontiguousarray(b_gi.reshape(1, C), dtype=np.float32)
    wtetaT = np.asarray(w_teta.T, dtype=np.float32)
    maps = []
    for c in range(8):
        b, r = c // 2, c % 2
        # rotate so the local q-half sits in columns 0:128, keep only it
        wtt_loc = np.roll(wtetaT, -128 * r, axis=1)[:, 0:128]
        wtg13 = np.ascontiguousarray(
            np.concatenate([wtt_loc, wtgT], axis=1), dtype=np.float16)
        maps.append({
            "x_full": xf16[b],
            "x_half": np.ascontiguousarray(xf[b][:, NH * r:NH * (r + 1)]),
            "wtg13": wtg13,
            "wtf": wtf, "wo": wo,
            "bt": np.ascontiguousarray(
                b_teta[128 * r:128 * (r + 1)].reshape(1, 128), dtype=np.float32),
            "bg": bg, "bf": bf, "bo1": bo1, "bo2": bo2,
        })
    return maps


def run(trace=False, **inputs):
    if "nc" not in _CACHE:
        _CACHE["nc"] = _build_nc()
    nc = _CACHE["nc"]
    maps = _in_maps(**inputs)
    res = run_bass_kernel_spmd(nc, maps, core_ids=list(range(8)), trace=trace)
    out = np.empty((B, 2 * C, N), dtype=np.float32)
    for c in range(8):
        b, r = c // 2, c % 2
        out[b][:, NH * r:NH * (r + 1)] = res.results[c]["out"]
    return out.reshape(B, 2 * C, H, W), res


def kernel(**inputs):
    out, _ = run(trace=False, **inputs)
    return out
